# revision 1
# baseline (speedup 1.0000x reference)
"""DKVMN forward kernel for Trainium2, 8-core batch-parallel.

Model (per sample): T=200 sequential DKVMN memory steps over state
Mv [M=64, D=64], with read-before-update, plus embedding gathers and
small projections around the recurrence.

Sharding: data-parallel over batch. 64 samples -> 8 cores x 8 samples.
Parameters (embedding tables + small weights) replicated per core.

Row space for the parallel phases is SAMPLE-MAJOR with per-sample pad:
row = b_loc*208 + t  (b_loc = g*2 + s; t in [0,208), pad t>=200).
8*208 = 1664 = 13 blocks of 128. Pad rows compute garbage that is
never read back. Sample-major keeps every (g,s)-slice t-contiguous,
so all layout shuffles are plain <=3-dim DMAs.

Per-core recurrence layout (state S[(s,d), (g,m)] = [128, 256] f32):
  per step t:
    Wbc[128,256] = PE broadcast of w_t (indicator matmul, exact fp32)
    P1 = S * Wbc                      (DVE)
    read4[:,g]   = sum_m P1           (DVE segmented reduce)
    S = S - P1 * e_t[d-bcast]         (DVE x2, step-0 AP broadcast)
    S = S + Wbc * a_t[d-bcast]        (DVE x2)
"""

import numpy as np

import concourse.bass as bass
import concourse.bacc as bacc
import concourse.tile as tile
from concourse import mybir
from concourse.bass_utils import run_bass_kernel_spmd

F32 = mybir.dt.float32
I32 = mybir.dt.int32
AL = mybir.AluOpType
AF = mybir.ActivationFunctionType
AX = mybir.AxisListType

NUM_SKILLS = 1000
D = 64          # dim_s
M = 64          # size_m
B = 64          # global batch
T = 200
TP = 208        # padded per-sample length
NCORES = 8
BL = B // NCORES          # samples per core = 8
RPAD = BL * TP            # 1664
NBLK = RPAD // 128        # 13
TCH = 25                  # scan-loop w-stream chunk (steps)
NCH = T // TCH            # 8


def _build():
    nc = bacc.Bacc(None, target_bir_lowering=False, debug=False)

    # ---- external I/O ----
    d_idxk = nc.dram_tensor("idxk", [RPAD], I32, kind="ExternalInput")
    d_resp = nc.dram_tensor("resp", [RPAD], I32, kind="ExternalInput")
    d_kemb = nc.dram_tensor("kemb", [NUM_SKILLS, D], F32, kind="ExternalInput")
    d_vemb = nc.dram_tensor("vemb", [2 * NUM_SKILLS, D], F32, kind="ExternalInput")
    d_mkt = nc.dram_tensor("mkt", [D, M], F32, kind="ExternalInput")      # Mk^T
    d_eawt = nc.dram_tensor("eawt", [D, 2 * D], F32, kind="ExternalInput")  # [eW^T|aW^T]
    d_fwt = nc.dram_tensor("fwt", [2 * D, D], F32, kind="ExternalInput")  # fW^T
    d_pwb = nc.dram_tensor("pwb", [128, D], F32, kind="ExternalInput")    # pW bcast
    d_ebc = nc.dram_tensor("ebc", [D, 1], F32, kind="ExternalInput")      # eb col
    d_abc = nc.dram_tensor("abc", [D, 1], F32, kind="ExternalInput")      # ab col
    d_fbb = nc.dram_tensor("fbb", [128, D], F32, kind="ExternalInput")    # fb bcast
    d_pbc = nc.dram_tensor("pbc", [128, 1], F32, kind="ExternalInput")    # pb col
    d_ind2 = nc.dram_tensor("ind2", [2, 128], F32, kind="ExternalInput")
    d_ident = nc.dram_tensor("ident", [128, 128], F32, kind="ExternalInput")
    d_s0 = nc.dram_tensor("s0", [128, 4 * M], F32, kind="ExternalInput")  # Mv0 prelaid
    d_out = nc.dram_tensor("out", [BL, T - 1], F32, kind="ExternalOutput")

    # ---- internal DRAM staging (w only) ----
    d_w = nc.dram_tensor("w_stg", [RPAD, M], F32)
    d_p = nc.dram_tensor("p_stg", [RPAD], F32)

    with tile.TileContext(nc) as tc:
        import contextlib
        with contextlib.ExitStack() as ctx:
            singles = ctx.enter_context(tc.tile_pool(name="singles", bufs=1))

            t_idxk = singles.tile([128, NBLK], I32)
            t_idxv = singles.tile([128, NBLK], I32)
            t_resp = singles.tile([128, NBLK], I32)
            t_mkt = singles.tile([D, M], F32)
            t_eawt = singles.tile([D, 2 * D], F32)
            t_fwt1 = singles.tile([D, D], F32)
            t_fwt2 = singles.tile([D, D], F32)
            t_pwb = singles.tile([128, D], F32)
            t_ebc = singles.tile([D, 1], F32)
            t_abc = singles.tile([D, 1], F32)
            t_fbb = singles.tile([128, D], F32)
            t_pbc = singles.tile([128, 1], F32)
            t_ind2 = singles.tile([2, 128], F32)
            t_ident = singles.tile([128, 128], F32)
            t_kT = singles.tile([D, RPAD], F32)      # k^T, sample-major cols
            t_eT = singles.tile([D, RPAD], F32)      # sigmoid(v@eW^T+eb)^T
            t_aT = singles.tile([D, RPAD], F32)      # tanh(v@aW^T+ab)^T
            t_rdT = singles.tile([D, RPAD], F32)     # reads^T for stage C
            t_S = singles.tile([128, 4 * M], F32)    # recurrence state
            t_E4 = singles.tile([128, 4, T], F32)    # e in [(s,d), g, t]
            t_A4 = singles.tile([128, 4, T], F32)
            t_reads = singles.tile([128, 4, T], F32)
            t_psig = singles.tile([128, NBLK], F32)

            nc.sync.dma_start(out=t_idxk[:], in_=d_idxk[:].rearrange("(c p) -> p c", p=128))
            nc.sync.dma_start(out=t_resp[:], in_=d_resp[:].rearrange("(c p) -> p c", p=128))
            nc.sync.dma_start(out=t_mkt[:], in_=d_mkt[:])
            nc.sync.dma_start(out=t_eawt[:], in_=d_eawt[:])
            nc.sync.dma_start(out=t_fwt1[:], in_=d_fwt[0:D, :])
            nc.sync.dma_start(out=t_fwt2[:], in_=d_fwt[D:2 * D, :])
            nc.sync.dma_start(out=t_pwb[:], in_=d_pwb[:])
            nc.sync.dma_start(out=t_ebc[:], in_=d_ebc[:])
            nc.sync.dma_start(out=t_abc[:], in_=d_abc[:])
            nc.sync.dma_start(out=t_fbb[:], in_=d_fbb[:])
            nc.sync.dma_start(out=t_pbc[:], in_=d_pbc[:])
            nc.sync.dma_start(out=t_ind2[:], in_=d_ind2[:])
            nc.sync.dma_start(out=t_ident[:], in_=d_ident[:])
            nc.sync.dma_start(out=t_S[:], in_=d_s0[:])

            # v-table index: x = skills + NUM_SKILLS * responses
            # (responses in {0,1}, so the >-1 mask in the reference is identity)
            nc.vector.tensor_scalar(out=t_idxv[:], in0=t_resp[:], scalar1=NUM_SKILLS,
                                    scalar2=None, op0=AL.mult)
            nc.vector.tensor_tensor(out=t_idxv[:], in0=t_idxv[:], in1=t_idxk[:], op=AL.add)

            # ================= stage A: gathers, w / e^T / a^T =================
            with tc.tile_pool(name="sa_sb", bufs=3) as sa, \
                 tc.tile_pool(name="sa_ps", bufs=2, space="PSUM") as sap:
                for c in range(NBLK):
                    kg = sa.tile([128, D], F32, tag="kg")
                    vg = sa.tile([128, D], F32, tag="vg")
                    nc.gpsimd.indirect_dma_start(
                        out=kg[:], out_offset=None, in_=d_kemb[:],
                        in_offset=bass.IndirectOffsetOnAxis(ap=t_idxk[:, c:c + 1], axis=0))
                    nc.gpsimd.indirect_dma_start(
                        out=vg[:], out_offset=None, in_=d_vemb[:],
                        in_offset=bass.IndirectOffsetOnAxis(ap=t_idxv[:, c:c + 1], axis=0))
                    kTp = sap.tile([D, 128], F32, tag="ktp", space="PSUM")
                    vTp = sap.tile([D, 128], F32, tag="vtp", space="PSUM")
                    nc.tensor.transpose(out=kTp[:], in_=kg[:], identity=t_ident[:])
                    nc.tensor.transpose(out=vTp[:], in_=vg[:], identity=t_ident[:])
                    nc.scalar.copy(out=t_kT[:, c * 128:(c + 1) * 128], in_=kTp[:])
                    vT = sa.tile([D, 128], F32, tag="vt")
                    nc.scalar.copy(out=vT[:], in_=vTp[:])

                    # logits = k @ Mk^T  -> [128 rows, M]
                    lg = sap.tile([128, M], F32, tag="lg", space="PSUM")
                    nc.tensor.matmul(lg[:], lhsT=t_kT[:, c * 128:(c + 1) * 128],
                                     rhs=t_mkt[:], start=True, stop=True)
                    mx = sa.tile([128, 1], F32, tag="mx")
                    nc.vector.tensor_reduce(out=mx[:], in_=lg[:], axis=AX.X,
                                            op=AL.max, negate=True)
                    wexp = sa.tile([128, M], F32, tag="wexp")
                    sme = sa.tile([128, 1], F32, tag="sme")
                    nc.scalar.activation(out=wexp[:], in_=lg[:], func=AF.Exp,
                                         bias=mx[:], scale=1.0, accum_out=sme[:])
                    rin = sa.tile([128, 1], F32, tag="rin")
                    nc.vector.reciprocal(out=rin[:], in_=sme[:])
                    wb = sa.tile([128, M], F32, tag="wb")
                    nc.vector.tensor_scalar(out=wb[:], in0=wexp[:], scalar1=rin[:],
                                            scalar2=None, op0=AL.mult)
                    nc.sync.dma_start(out=d_w[c * 128:(c + 1) * 128, :], in_=wb[:])

                    # e/a transposed: eaT = [eW^T|aW^T]^T @ v^T -> [(e|a), rows]
                    eaT = sap.tile([2 * D, 128], F32, tag="eat", space="PSUM")
                    nc.tensor.matmul(eaT[:], lhsT=t_eawt[:], rhs=vT[:],
                                     start=True, stop=True)
                    nc.scalar.activation(out=t_eT[:, c * 128:(c + 1) * 128],
                                         in_=eaT[0:D, :], func=AF.Sigmoid,
                                         bias=t_ebc[:], scale=1.0)
                    nc.scalar.activation(out=t_aT[:, c * 128:(c + 1) * 128],
                                         in_=eaT[D:2 * D, :], func=AF.Tanh,
                                         bias=t_abc[:], scale=1.0)

            # ============ stage A2: (s,d)-packed e/a tiles ============
            for g in range(4):
                for s in range(2):
                    col = (g * 2 + s) * TP
                    nc.sync.dma_start(out=t_E4[s * D:(s + 1) * D, g, 0:T],
                                      in_=t_eT[:, col:col + T])
                    nc.sync.dma_start(out=t_A4[s * D:(s + 1) * D, g, 0:T],
                                      in_=t_aT[:, col:col + T])

            # negate e for the gate trick: G = 1 + Wbc*(-e)
            nc.vector.tensor_scalar(out=t_E4[:].rearrange("p g t -> p (g t)"),
                                    in0=t_E4[:].rearrange("p g t -> p (g t)"),
                                    scalar1=-1.0, scalar2=None, op0=AL.mult)

            # ================= stage B: the recurrence =================
            import os as _os
            _nch = int(_os.environ.get("BSTEPS", str(T))) // TCH
            with tc.tile_pool(name="sb_w", bufs=2) as sbw, \
                 tc.tile_pool(name="sb_t", bufs=3) as sbt, \
                 tc.tile_pool(name="sb_ps", bufs=4, space="PSUM") as sbp:
                for ch in range(_nch):
                    wch = sbw.tile([2, TCH, 4, M], F32, tag="wch")
                    for g in range(4):
                        nc.sync.dma_start(
                            out=wch[:, :, g, :],
                            in_=d_w[:].rearrange("(b t) m -> b t m", b=BL)[
                                g * 2:g * 2 + 2, ch * TCH:(ch + 1) * TCH, :])
                    for tt in range(TCH):
                        t = ch * TCH + tt
                        wbc = sbp.tile([128, 4 * M], F32, tag="wbc", space="PSUM")
                        nc.tensor.matmul(
                            wbc[:], lhsT=t_ind2[:],
                            rhs=wch[:, tt, :, :].rearrange("s g m -> s (g m)"),
                            start=True, stop=True)
                        wbc_v = wbc[:].rearrange("p (g m) -> p g m", g=4)
                        # ACT: gate G = 1 - Wbc*e  (per-g, scale is [P,1])
                        gt = sbt.tile([128, 4, M], F32, tag="gt")
                        for g in range(4):
                            nc.scalar.activation(
                                out=gt[:, g, :], in_=wbc_v[:, g, :], func=AF.Copy,
                                bias=1.0, scale=t_E4[:, g, t:t + 1])
                        # DVE: read product + segmented reduce, then apply update
                        p1 = sbt.tile([128, 4 * M], F32, tag="p1")
                        nc.vector.tensor_tensor(out=p1[:], in0=t_S[:], in1=wbc[:], op=AL.mult)
                        nc.vector.tensor_reduce(
                            out=t_reads[:, :, t],
                            in_=p1[:].rearrange("p (g m) -> p g m", g=4),
                            axis=AX.X, op=AL.add)
                        nc.vector.tensor_tensor(
                            out=t_S[:], in0=t_S[:],
                            in1=gt[:].rearrange("p g m -> p (g m)"), op=AL.mult)
                        t2 = sbt.tile([128, 4, M], F32, tag="t2")
                        nc.vector.tensor_tensor(
                            out=t2[:],
                            in0=wbc_v[:],
                            in1=t_A4[:, :, t].unsqueeze(2).broadcast_to([128, 4, M]),
                            op=AL.mult)
                        nc.vector.tensor_tensor(
                            out=t_S[:], in0=t_S[:],
                            in1=t2[:].rearrange("p g m -> p (g m)"), op=AL.add)

            # reads -> [D, RPAD] sample-major for stage C
            for g in range(4):
                for s in range(2):
                    col = (g * 2 + s) * TP
                    nc.sync.dma_start(out=t_rdT[:, col:col + T],
                                      in_=t_reads[s * D:(s + 1) * D, g, 0:T])

            # ================= stage C: output head =================
            with tc.tile_pool(name="sc_sb", bufs=3) as sc, \
                 tc.tile_pool(name="sc_ps", bufs=2, space="PSUM") as scp:
                for c in range(NBLK):
                    fp = scp.tile([128, D], F32, tag="fp", space="PSUM")
                    nc.tensor.matmul(fp[:], lhsT=t_rdT[:, c * 128:(c + 1) * 128],
                                     rhs=t_fwt1[:], start=True, stop=False)
                    nc.tensor.matmul(fp[:], lhsT=t_kT[:, c * 128:(c + 1) * 128],
                                     rhs=t_fwt2[:], start=False, stop=True)
                    fb = sc.tile([128, D], F32, tag="fb")
                    nc.vector.tensor_tensor(out=fb[:], in0=fp[:], in1=t_fbb[:], op=AL.add)
                    ft = sc.tile([128, D], F32, tag="ft")
                    nc.scalar.activation(out=ft[:], in_=fb[:], func=AF.Tanh)
                    junk = sc.tile([128, D], F32, tag="junk")
                    nc.vector.scalar_tensor_tensor(
                        out=junk[:], in0=ft[:], scalar=1.0, in1=t_pwb[:],
                        op0=AL.mult, op1=AL.mult,
                        accum_out=t_psig[:, c:c + 1])
                nc.scalar.activation(out=t_psig[:], in_=t_psig[:], func=AF.Sigmoid,
                                     bias=t_pbc[:], scale=1.0)
                nc.sync.dma_start(out=d_p[:].rearrange("(c p) -> p c", p=128), in_=t_psig[:])

                # out[b, j] = p[b*208 + 1 + j]
                ob = sc.tile([BL, T - 1], F32, tag="ob")
                nc.sync.dma_start(
                    out=ob[:],
                    in_=d_p[:].rearrange("(b t) -> b t", b=BL)[:, 1:T])
                nc.sync.dma_start(out=d_out[:], in_=ob[:])

    nc.compile()
    return nc


_NC_CACHE = None


def _get_nc():
    global _NC_CACHE
    if _NC_CACHE is None:
        _NC_CACHE = _build()
    return _NC_CACHE


def kernel(skills, responses, k_emb, v_emb, Mk, Mv0, fW, fb, eW, eb, aW, ab, pW, pb):
    skills = np.asarray(skills)
    responses = np.asarray(responses)
    k_emb = np.asarray(k_emb, dtype=np.float32)
    v_emb = np.asarray(v_emb, dtype=np.float32)
    Mk = np.asarray(Mk, dtype=np.float32)
    Mv0 = np.asarray(Mv0, dtype=np.float32)
    fW = np.asarray(fW, dtype=np.float32)
    fb = np.asarray(fb, dtype=np.float32)
    eW = np.asarray(eW, dtype=np.float32)
    eb = np.asarray(eb, dtype=np.float32)
    aW = np.asarray(aW, dtype=np.float32)
    ab = np.asarray(ab, dtype=np.float32)
    pW = np.asarray(pW, dtype=np.float32)
    pb = np.asarray(pb, dtype=np.float32)

    mkt = np.ascontiguousarray(Mk.T)                                   # [D, M]
    eawt = np.ascontiguousarray(np.concatenate([eW.T, aW.T], axis=1))  # [D, 2D]
    fwt = np.ascontiguousarray(fW.T)                                   # [2D, D]
    pwb = np.broadcast_to(pW, (128, D)).copy()
    ebc = np.ascontiguousarray(eb.reshape(D, 1))
    abc = np.ascontiguousarray(ab.reshape(D, 1))
    fbb = np.broadcast_to(fb[None, :], (128, D)).copy()
    pbc = np.full((128, 1), float(pb[0]), np.float32)
    ind2 = np.zeros((2, 128), np.float32)
    ind2[0, :64] = 1.0
    ind2[1, 64:] = 1.0
    ident = np.eye(128, dtype=np.float32)
    # S0[(s,d),(g,m)] = Mv0[m,d]
    s0 = np.tile(Mv0.T.reshape(1, D, 1, M), (2, 1, 4, 1)).reshape(128, 4 * M)
    s0 = np.ascontiguousarray(s0, dtype=np.float32)

    shared = dict(kemb=k_emb, vemb=v_emb, mkt=mkt, eawt=eawt, fwt=fwt,
                  pwb=pwb, ebc=ebc, abc=abc, fbb=fbb, pbc=pbc, ind2=ind2,
                  ident=ident, s0=s0)

    in_maps = []
    for core in range(NCORES):
        sk = skills[core * BL:(core + 1) * BL].astype(np.int32)
        rs = responses[core * BL:(core + 1) * BL].astype(np.int32)
        idxk = np.zeros((BL, TP), np.int32)
        resp = np.zeros((BL, TP), np.int32)
        idxk[:, :T] = sk          # row = b*208 + t
        resp[:, :T] = rs
        m = dict(shared)
        m["idxk"] = idxk.reshape(-1)
        m["resp"] = resp.reshape(-1)
        in_maps.append(m)

    nc = _get_nc()
    res = run_bass_kernel_spmd(nc, in_maps, core_ids=list(range(NCORES)),
                               **_RUN_KWARGS)
    out = np.concatenate([res.results[i]["out"] for i in range(NCORES)], axis=0)
    global _LAST_RESULT
    _LAST_RESULT = res
    return out.astype(np.float32)


_RUN_KWARGS = {}
_LAST_RESULT = None


def run_traced(**inputs):
    """Run once with NTFF tracing; returns exec_time_ns (or None)."""
    global _RUN_KWARGS
    _RUN_KWARGS = {"trace": True}
    try:
        kernel(**inputs)
    finally:
        _RUN_KWARGS = {}
    return _LAST_RESULT.exec_time_ns if _LAST_RESULT is not None else None



# revision 13
# speedup vs baseline: 1.2525x; 1.2525x over previous
"""DKVMN forward kernel for Trainium2, 8-core batch-parallel, scan-based.

Model (per sample): T=200 sequential DKVMN memory steps over state
Mv [M=64, D=64], with read-before-update, plus embedding gathers and
small projections around the recurrence.

Sharding: data-parallel over batch. 64 samples -> 8 cores x 8 samples.
Parameters replicated per core.

Row space is TIME-MAJOR in 128-row blocks: block c covers timesteps
t in [c*16, (c+1)*16) for all 8 local samples:
  row = c*128 + b_loc*16 + tt,  b_loc = g*2 + s in [0,8), tt in [0,16).
13 blocks = 208 padded timesteps (t >= 200 is pad, computed, unread).
Stage-A block c feeds stage-B chunk c directly -> full pipeline overlap.

Recurrence (per core, partitions (s,d) = 128, free (g,m)=256 per step):
  S_t = S_{t-1} * (1 - w_t (x) e_t) + w_t (x) a_t ;  read_t = sum_m w*S_{t-1}
run as a *batched segmented scan*: per chunk of C=16 steps,
  GT[(s,d),(g,m,t)] = 1 - wbc*e   (w broadcast via PE matmul, fp16)
  UT              = wbc*a
  GT[...,0]=0, UT[...,0]=GT0*S_prev+UT0   (segment reset carries state)
  SALL = tensor_tensor_scan(GT, UT)       (state fp32 internal)
  P1 = SALL(shift 1) * wbc ; reads = reduce_m(P1)
"""

import numpy as np

import concourse.bass as bass
import concourse.bacc as bacc
import concourse.tile as tile
from concourse import mybir
from concourse.bass_utils import run_bass_kernel_spmd

F32 = mybir.dt.float32
F16 = mybir.dt.float16
BF16 = mybir.dt.bfloat16
I32 = mybir.dt.int32
AL = mybir.AluOpType
AF = mybir.ActivationFunctionType
AX = mybir.AxisListType

NUM_SKILLS = 1000
D = 64          # dim_s
M = 64          # size_m
B = 64          # global batch
T = 200
NCORES = 8
BL = B // NCORES          # samples per core = 8
C = 16                    # timesteps per chunk / block
NBLK = 13                 # 13 blocks of 128 rows = 208 padded steps
TP = NBLK * C             # 208
RPAD = NBLK * 128         # 1664


def _build():
    import os
    stages = os.environ.get("KSTAGES", "ABC")
    nc = bacc.Bacc(None, target_bir_lowering=False, debug=False)

    # ---- external I/O ----
    d_idxk = nc.dram_tensor("idxk", [RPAD], I32, kind="ExternalInput")
    d_resp = nc.dram_tensor("resp", [RPAD], I32, kind="ExternalInput")
    d_kemb = nc.dram_tensor("kemb", [NUM_SKILLS, D], F32, kind="ExternalInput")
    d_vemb = nc.dram_tensor("vemb", [2 * NUM_SKILLS, D], F32, kind="ExternalInput")
    d_mkt = nc.dram_tensor("mkt", [D, M], BF16, kind="ExternalInput")       # Mk^T
    d_eawt = nc.dram_tensor("eawt", [D, 2 * D], BF16, kind="ExternalInput")  # [eW^T|aW^T]
    d_fwt1 = nc.dram_tensor("fwt1", [128, D], BF16, kind="ExternalInput")   # fW^T rows 0:64, x2
    d_fwt2 = nc.dram_tensor("fwt2", [128, D], BF16, kind="ExternalInput")   # fW^T rows 64:128, x2
    d_pwc = nc.dram_tensor("pwc", [128, 1], BF16, kind="ExternalInput")     # pW col x2
    d_ebc = nc.dram_tensor("ebc", [D, 1], F32, kind="ExternalInput")        # eb col
    d_abc = nc.dram_tensor("abc", [D, 1], F32, kind="ExternalInput")        # ab col
    d_fbc = nc.dram_tensor("fbc", [D, 1], F32, kind="ExternalInput")        # fb col
    d_ind8 = nc.dram_tensor("ind8", [8, 4 * 128], F16, kind="ExternalInput")
    d_ident = nc.dram_tensor("ident", [128, 128], F32, kind="ExternalInput")
    d_s0 = nc.dram_tensor("s0", [128, 4 * M], F16, kind="ExternalInput")    # Mv0 prelaid
    d_out = nc.dram_tensor("out", [BL, T - 1], F32, kind="ExternalOutput")

    pb_host = _PB[0]

    with tile.TileContext(nc) as tc:
        import contextlib
        with contextlib.ExitStack() as ctx:
            singles = ctx.enter_context(tc.tile_pool(name="singles", bufs=1))

            t_idxk = singles.tile([128, NBLK], I32)
            t_idxv = singles.tile([128, NBLK], I32)
            t_resp = singles.tile([128, NBLK], I32)
            t_mkt = singles.tile([D, M], BF16)
            t_eawt = singles.tile([D, 2 * D], BF16)
            t_fwt1 = singles.tile([128, D], BF16)
            t_fwt2 = singles.tile([128, D], BF16)
            t_pwc = singles.tile([128, 1], BF16)
            t_ebc = singles.tile([D, 1], F32)
            t_abc = singles.tile([D, 1], F32)
            t_fbc = singles.tile([D, 1], F32)
            t_ind8 = singles.tile([8, 4 * 128], F16)
            t_ident = singles.tile([128, 128], F32)
            t_s0 = singles.tile([128, 4 * M], F16)
            t_kT = singles.tile([D, RPAD], BF16)       # k^T, block-row cols
            t_eT = singles.tile([D, RPAD], F16)        # sigmoid(e) pre-shuffle
            t_aT = singles.tile([D, RPAD], F16)        # tanh(a) pre-shuffle
            t_E4 = singles.tile([128, 4, TP], F16)     # e in [(s,d), g, t]
            t_A4 = singles.tile([128, 4, TP], F16)
            t_WS = singles.tile([BL, M * TP], F16)     # w, [b_loc, (t,m)]
            t_reads = singles.tile([128, 4, TP], F32)
            t_rb0 = singles.tile([64, 4, TP], BF16)
            t_rb1 = singles.tile([64, 4, TP], BF16)
            t_psig = singles.tile([BL, TP], F32)
            # chunk-carried scan state (alternate buffers)
            t_sall0 = singles.tile([128, 4, M, C], F16)
            t_sall1 = singles.tile([128, 4, M, C], F16)
            t_sall = [t_sall0, t_sall1]

            nc.sync.dma_start(out=t_idxk[:], in_=d_idxk[:].rearrange("(c p) -> p c", p=128))
            nc.sync.dma_start(out=t_resp[:], in_=d_resp[:].rearrange("(c p) -> p c", p=128))
            nc.sync.dma_start(out=t_mkt[:], in_=d_mkt[:])
            nc.sync.dma_start(out=t_eawt[:], in_=d_eawt[:])
            nc.sync.dma_start(out=t_fwt1[:], in_=d_fwt1[:])
            nc.sync.dma_start(out=t_fwt2[:], in_=d_fwt2[:])
            nc.sync.dma_start(out=t_pwc[:], in_=d_pwc[:])
            nc.sync.dma_start(out=t_ebc[:], in_=d_ebc[:])
            nc.sync.dma_start(out=t_abc[:], in_=d_abc[:])
            nc.sync.dma_start(out=t_fbc[:], in_=d_fbc[:])
            nc.sync.dma_start(out=t_ind8[:], in_=d_ind8[:])
            nc.sync.dma_start(out=t_ident[:], in_=d_ident[:])
            nc.sync.dma_start(out=t_s0[:], in_=d_s0[:])

            # v-table index: x = skills + NUM_SKILLS * responses
            nc.vector.tensor_scalar(out=t_idxv[:], in0=t_resp[:], scalar1=NUM_SKILLS,
                                    scalar2=None, op0=AL.mult)
            nc.vector.tensor_tensor(out=t_idxv[:], in0=t_idxv[:], in1=t_idxk[:], op=AL.add)

            with tc.tile_pool(name="sa_sb", bufs=3) as sa, \
                 tc.tile_pool(name="sa_ps", bufs=1, space="PSUM") as sap, \
                 tc.tile_pool(name="sb_sb", bufs=2) as sb, \
                 tc.tile_pool(name="sb_ps", bufs=4, space="PSUM") as sbp:
                for c in range(NBLK):
                    # ============ stage A: gathers, w / e / a for block c ============
                    kg = sa.tile([128, D], F32, tag="kg")
                    vg = sa.tile([128, D], F32, tag="vg")
                    nc.gpsimd.indirect_dma_start(
                        out=kg[:], out_offset=None, in_=d_kemb[:],
                        in_offset=bass.IndirectOffsetOnAxis(ap=t_idxk[:, c:c + 1], axis=0))
                    nc.gpsimd.indirect_dma_start(
                        out=vg[:], out_offset=None, in_=d_vemb[:],
                        in_offset=bass.IndirectOffsetOnAxis(ap=t_idxv[:, c:c + 1], axis=0))
                    kTp = sap.tile([D, 128], F32, tag="ktp", space="PSUM")
                    vTp = sap.tile([D, 128], F32, tag="vtp", space="PSUM")
                    nc.tensor.transpose(out=kTp[:], in_=kg[:], identity=t_ident[:])
                    nc.tensor.transpose(out=vTp[:], in_=vg[:], identity=t_ident[:])
                    nc.scalar.copy(out=t_kT[:, c * 128:(c + 1) * 128], in_=kTp[:])
                    vT = sa.tile([D, 128], BF16, tag="vt")
                    nc.scalar.copy(out=vT[:], in_=vTp[:])

                    # logits = k @ Mk^T -> [128 rows, M]
                    lg = sap.tile([128, M], F32, tag="lg", space="PSUM")
                    nc.tensor.matmul(lg[:], lhsT=t_kT[:, c * 128:(c + 1) * 128],
                                     rhs=t_mkt[:], start=True, stop=True)
                    mx = sa.tile([128, 1], F32, tag="mx")
                    nc.vector.tensor_reduce(out=mx[:], in_=lg[:], axis=AX.X,
                                            op=AL.max, negate=True)
                    wexp = sa.tile([128, M], F32, tag="wexp")
                    sme = sa.tile([128, 1], F32, tag="sme")
                    nc.scalar.activation(out=wexp[:], in_=lg[:], func=AF.Exp,
                                         bias=mx[:], scale=1.0, accum_out=sme[:])
                    rin = sa.tile([128, 1], F32, tag="rin")
                    nc.vector.reciprocal(out=rin[:], in_=sme[:])
                    wb = sa.tile([128, M], F16, tag="wb")
                    nc.vector.tensor_scalar(out=wb[:], in0=wexp[:], scalar1=rin[:],
                                            scalar2=None, op0=AL.mult)
                    # w rows -> s-major scan layout WS[b_loc, (t, m)]
                    for b_loc in range(BL):
                        nc.sync.dma_start(
                            out=t_WS[b_loc:b_loc + 1, (c * C) * M:(c * C + C) * M],
                            in_=wb[b_loc * C:(b_loc + 1) * C, :])

                    # e/a: eaT = [eW^T|aW^T]^T @ v^T -> [(e|a)d, rows]
                    eaT = sap.tile([2 * D, 128], F32, tag="eat", space="PSUM")
                    nc.tensor.matmul(eaT[:], lhsT=t_eawt[:], rhs=vT[:],
                                     start=True, stop=True)
                    nc.scalar.activation(out=t_eT[:, c * 128:(c + 1) * 128],
                                         in_=eaT[0:D, :], func=AF.Sigmoid,
                                         bias=t_ebc[:], scale=1.0)
                    nc.scalar.activation(out=t_aT[:, c * 128:(c + 1) * 128],
                                         in_=eaT[D:2 * D, :], func=AF.Tanh,
                                         bias=t_abc[:], scale=1.0)
                    # shuffle into per-(s,g) t-contiguous layout
                    for g in range(4):
                        for s in range(2):
                            col = c * 128 + (g * 2 + s) * C
                            nc.sync.dma_start(out=t_E4[s * D:(s + 1) * D, g, c * C:(c + 1) * C],
                                              in_=t_eT[:, col:col + C])
                            nc.sync.dma_start(out=t_A4[s * D:(s + 1) * D, g, c * C:(c + 1) * C],
                                              in_=t_aT[:, col:col + C])

                    # ============ stage B: chunk c (timesteps c*16 .. +16) ============
                    if "B" not in stages:
                        continue
                    wbc = sb.tile([128, 4, M, C], F16, tag="wbc")
                    for h in range(2):
                        for g in range(4):
                            wps = sbp.tile([128, 8 * M], F32, tag="wps", space="PSUM")
                            nc.tensor.matmul(
                                wps[:],
                                lhsT=t_ind8[:, g * 128:(g + 1) * 128],
                                rhs=t_WS[:, (c * C + h * 8) * M:(c * C + h * 8 + 8) * M],
                                start=True, stop=True)
                            # PSUM (t,m) -> SBUF fp16 (m,t)
                            nc.scalar.copy(
                                out=wbc[:, g, :, h * 8:(h + 1) * 8],
                                in_=wps[:].rearrange("p (t m) -> p m t", t=8))

                    ebc4 = t_E4[:, :, c * C:(c + 1) * C].unsqueeze(2).broadcast_to([128, 4, M, C])
                    abc4 = t_A4[:, :, c * C:(c + 1) * C].unsqueeze(2).broadcast_to([128, 4, M, C])

                    prod = sb.tile([128, 4, M, C], F16, tag="prod")
                    nc.vector.tensor_tensor(out=prod[:], in0=wbc[:], in1=ebc4, op=AL.mult)
                    gt = sb.tile([128, 4, M, C], F16, tag="gt")
                    nc.scalar.activation(
                        out=gt[:].rearrange("p g m t -> p (g m t)"),
                        in_=prod[:].rearrange("p g m t -> p (g m t)"),
                        func=AF.Copy, bias=1.0, scale=-1.0)
                    ut = sb.tile([128, 4, M, C], F16, tag="ut")
                    nc.vector.tensor_tensor(out=ut[:], in0=wbc[:], in1=abc4, op=AL.mult)

                    # segment-boundary fix: carry state S_prev into t=0 slot
                    sprev = t_s0[:].rearrange("p (g m) -> p g m", g=4) if c == 0 \
                        else t_sall[(c - 1) % 2][:, :, :, C - 1]
                    btmp = sb.tile([128, 4, M], F16, tag="btmp")
                    nc.vector.tensor_tensor(out=btmp[:], in0=gt[:, :, :, 0], in1=sprev,
                                            op=AL.mult)
                    nc.vector.tensor_tensor(out=ut[:, :, :, 0], in0=ut[:, :, :, 0],
                                            in1=btmp[:], op=AL.add)
                    nc.vector.memset(gt[:, :, :, 0], 0.0)

                    # the scan: S_t = GT_t * S_{t-1} + UT_t along flat (g,m,t)
                    sall = t_sall[c % 2]
                    nc.vector.tensor_tensor_scan(
                        out=sall[:].rearrange("p g m t -> p (g m t)"),
                        data0=gt[:].rearrange("p g m t -> p (g m t)"),
                        data1=ut[:].rearrange("p g m t -> p (g m t)"),
                        initial=0.0, op0=AL.mult, op1=AL.add)

                    # reads: P1_t = S_{t-1} * wbc_t ; reduce over m
                    p1 = sb.tile([128, 4, M, C], F16, tag="p1")
                    nc.vector.tensor_tensor(out=p1[:, :, :, 1:C], in0=sall[:, :, :, 0:C - 1],
                                            in1=wbc[:, :, :, 1:C], op=AL.mult)
                    nc.vector.tensor_tensor(out=p1[:, :, :, 0], in0=sprev,
                                            in1=wbc[:, :, :, 0], op=AL.mult)
                    nc.vector.tensor_reduce(
                        out=t_reads[:, :, c * C:(c + 1) * C],
                        in_=p1[:].rearrange("p g m t -> p g t m"),
                        axis=AX.X, op=AL.add)
                    nc.scalar.copy(out=t_rb0[:, :, c * C:(c + 1) * C],
                                   in_=t_reads[0:64, :, c * C:(c + 1) * C])
                    nc.scalar.copy(out=t_rb1[:, :, c * C:(c + 1) * C],
                                   in_=t_reads[64:128, :, c * C:(c + 1) * C])

            # ============ stage C: output head, per (s,g) ============
            with tc.tile_pool(name="sc_sb", bufs=2) as sc, \
                 tc.tile_pool(name="sc_ps", bufs=2, space="PSUM") as scp:
                if "C" not in stages:
                    nc.vector.memset(t_psig[:], 0.5)
                    if "B" not in stages:
                        nc.vector.memset(t_reads[:].rearrange("p g t -> p (g t)"), 0.0)
                for s in range(2):
                    if "C" not in stages:
                        break
                    for j in range(2):  # g-pairs {2j, 2j+1}
                        # two g-groups of this s at once: [64, 2*208]
                        fps = scp.tile([D, 2 * TP], F32, tag="fps", space="PSUM")
                        # k cols for b_loc = g*2+s, g in {2j, 2j+1}
                        kslice = t_kT[:].rearrange(
                            "p (c gg w) -> p gg c w", c=NBLK, gg=8)[
                            :, 4 * j + s:4 * j + s + 3:2, :, :]
                        t_rb = t_rb0 if s == 0 else t_rb1
                        nc.tensor.matmul(fps[:], lhsT=t_fwt1[0:D, :],
                                         rhs=t_rb[:, 2 * j:2 * j + 2, :],
                                         start=True, stop=False)
                        nc.tensor.matmul(fps[:], lhsT=t_fwt2[0:D, :],
                                         rhs=kslice,
                                         start=False, stop=True)
                        ft = sc.tile([D, 2 * TP], BF16, tag="ft")
                        nc.scalar.activation(out=ft[:], in_=fps[:], func=AF.Tanh,
                                             bias=t_fbc[:], scale=1.0)
                        pps = scp.tile([1, 2 * TP], F32, tag="pps", space="PSUM")
                        nc.tensor.matmul(pps[:], lhsT=t_pwc[0:D, :], rhs=ft[:],
                                         start=True, stop=True)
                        prow = sc.tile([1, 2 * TP], F32, tag="prow")
                        nc.scalar.activation(out=prow[:],
                                             in_=pps[:], func=AF.Sigmoid,
                                             bias=pb_host, scale=1.0)
                        for gg in range(2):
                            nc.sync.dma_start(
                                out=t_psig[(2 * j + gg) * 2 + s:
                                           (2 * j + gg) * 2 + s + 1, :],
                                in_=prow[:, gg * TP:(gg + 1) * TP])
                nc.sync.dma_start(out=d_out[:], in_=t_psig[:, 1:T])

    nc.compile()
    return nc


_NC_CACHE = None
_PB = [0.0]


def _get_nc():
    global _NC_CACHE
    if _NC_CACHE is None:
        _NC_CACHE = _build()
    return _NC_CACHE


def kernel(skills, responses, k_emb, v_emb, Mk, Mv0, fW, fb, eW, eb, aW, ab, pW, pb):
    skills = np.asarray(skills)
    responses = np.asarray(responses)
    k_emb = np.asarray(k_emb, dtype=np.float32)
    v_emb = np.asarray(v_emb, dtype=np.float32)
    Mk = np.asarray(Mk, dtype=np.float32)
    Mv0 = np.asarray(Mv0, dtype=np.float32)
    fW = np.asarray(fW, dtype=np.float32)
    fb = np.asarray(fb, dtype=np.float32)
    eW = np.asarray(eW, dtype=np.float32)
    eb = np.asarray(eb, dtype=np.float32)
    aW = np.asarray(aW, dtype=np.float32)
    ab = np.asarray(ab, dtype=np.float32)
    pW = np.asarray(pW, dtype=np.float32)
    pb = np.asarray(pb, dtype=np.float32)

    _PB[0] = float(pb[0])

    import ml_dtypes
    bf = ml_dtypes.bfloat16
    mkt = np.ascontiguousarray(Mk.T).astype(bf)                         # [D, M]
    eawt = np.ascontiguousarray(np.concatenate([eW.T, aW.T], axis=1)).astype(bf)
    fwt = np.ascontiguousarray(fW.T)                                    # [2D, D]
    fwt1 = np.vstack([fwt[0:D, :], fwt[0:D, :]]).astype(bf)
    fwt2 = np.vstack([fwt[D:2 * D, :], fwt[D:2 * D, :]]).astype(bf)
    pwc = np.vstack([pW.reshape(D, 1), pW.reshape(D, 1)]).astype(bf)
    ebc = np.ascontiguousarray(eb.reshape(D, 1))
    abc = np.ascontiguousarray(ab.reshape(D, 1))
    fbc = np.ascontiguousarray(fb.reshape(D, 1))
    ind8 = np.zeros((4, 8, 4, 128), np.float16)
    for g in range(4):
        for s in range(2):
            ind8[g, g * 2 + s, g, s * 64:(s + 1) * 64] = 1.0
    # layout [8, 4*128]: t_ind8[:, g*128:(g+1)*128] = selector for group g
    ind8 = np.ascontiguousarray(ind8.sum(axis=0).reshape(8, 4 * 128))
    ident = np.eye(128, dtype=np.float32)
    # S0[(s,d),(g,m)] = Mv0[m,d]
    s0 = np.tile(Mv0.T.reshape(1, D, 1, M), (2, 1, 4, 1)).reshape(128, 4 * M)
    s0 = np.ascontiguousarray(s0).astype(np.float16)

    shared = dict(kemb=k_emb, vemb=v_emb, mkt=mkt, eawt=eawt, fwt1=fwt1,
                  fwt2=fwt2, pwc=pwc, ebc=ebc, abc=abc, fbc=fbc, ind8=ind8,
                  ident=ident, s0=s0)

    in_maps = []
    for core in range(NCORES):
        sk = skills[core * BL:(core + 1) * BL].astype(np.int32)
        rs = responses[core * BL:(core + 1) * BL].astype(np.int32)
        # time-major padded layout: row = c*128 + b_loc*16 + tt, t = c*16+tt
        idxk = np.zeros((BL, TP), np.int32)
        resp = np.zeros((BL, TP), np.int32)
        idxk[:, :T] = sk
        resp[:, :T] = rs
        # [b, (c, tt)] -> [(c, b, tt)]
        idxk = idxk.reshape(BL, NBLK, C).transpose(1, 0, 2).reshape(-1)
        resp = resp.reshape(BL, NBLK, C).transpose(1, 0, 2).reshape(-1)
        m = dict(shared)
        m["idxk"] = np.ascontiguousarray(idxk)
        m["resp"] = np.ascontiguousarray(resp)
        in_maps.append(m)

    nc = _get_nc()
    res = run_bass_kernel_spmd(nc, in_maps, core_ids=list(range(NCORES)),
                               **_RUN_KWARGS)
    out = np.concatenate([res.results[i]["out"] for i in range(NCORES)], axis=0)
    global _LAST_RESULT
    _LAST_RESULT = res
    return out.astype(np.float32)


_RUN_KWARGS = {}
_LAST_RESULT = None


def run_traced(**inputs):
    """Run once with NTFF tracing; returns exec_time_ns (or None)."""
    global _RUN_KWARGS
    _RUN_KWARGS = {"trace": True}
    try:
        kernel(**inputs)
    finally:
        _RUN_KWARGS = {}
    return _LAST_RESULT.exec_time_ns if _LAST_RESULT is not None else None
